# revision 9
# baseline (speedup 1.0000x reference)
"""Trainium2 Bass kernel for grouped (neighborhood) multi-head attention.

Problem: B=2, N=8192, D=512, H=8 heads (d_k=64), K=32 neighbors/node.
  Q/K/V = x @ W{q,k,v}.T ; per-head LayerNorm on Q,K ; gather K,V rows at
  idx[n,k]; softmax(QK/sqrt(dk)) ; out = attn@Vg ; out @ Wout.T + bout.

Sharding (8 cores): core c owns batch b=c//4, node quarter q=c%4 (2048
nodes). Each core projects its own 2048 rows (fp32 matmul on PE), applies
per-head LN, casts K,V to bf16 and AllGathers the K|V rows within its
4-core batch group into Shared DRAM. The grouped attention then gathers
2KB bf16 K|V rows with indirect DMA and runs scores/softmax/AV on the
Vector engine using broadcast (0-step) access patterns. Out-projection
transposes the attention output via PE identity matmuls and finishes with
fp32 matmuls (+bias via a ones-row matmul).
"""

import sys

sys.path.insert(0, "/opt/trn_rl_repo")

import numpy as np
from contextlib import ExitStack

import concourse.bass as bass
import concourse.mybir as mybir
import concourse.tile as tile
from concourse import bacc
from concourse.bass import ts
from concourse.masks import make_identity

F32 = mybir.dt.float32
BF16 = mybir.dt.bfloat16
I32 = mybir.dt.int32

H = 8
DK = 64
D = 512
KN = 32
B = 2
NCORES = 8
LN_EPS = 1e-5
DCH = D // 128  # contraction chunks (4)


def build_nc(NB, NSH, KG=16):
    """Build the SPMD Bass program. NB = nodes per batch, NSH = nodes per
    core (NB // 4), KG = neighbor group size for gather/compute pipelining."""
    T = NSH // 128          # node tiles per core
    G = KN // KG            # neighbor groups
    CPB = NCORES // B       # cores per batch group (4)
    groups = [list(range(g * CPB, (g + 1) * CPB)) for g in range(B)]

    nc = bacc.Bacc(
        "TRN2", target_bir_lowering=False, debug=False, num_devices=NCORES
    )

    xT = nc.dram_tensor("xT_sh", [D, NSH], F32, kind="ExternalInput")
    idx = nc.dram_tensor("idx_sh", [NSH, KN], I32, kind="ExternalInput")
    WqT = nc.dram_tensor("WqT", [D, D], F32, kind="ExternalInput")
    WkT = nc.dram_tensor("WkT", [D, D], F32, kind="ExternalInput")
    WvT = nc.dram_tensor("WvT", [D, D], F32, kind="ExternalInput")
    WoT = nc.dram_tensor("WoutT", [D, D], F32, kind="ExternalInput")
    bout = nc.dram_tensor("bout", [1, D], F32, kind="ExternalInput")
    out = nc.dram_tensor("out_sh", [NSH, D], F32, kind="ExternalOutput")

    kv_shard = nc.dram_tensor("kv_shard", [NSH, 2 * D], BF16)
    kv_full = nc.dram_tensor("kv_full", [NB, 2 * D], BF16)

    with ExitStack() as ctx:
        tc = ctx.enter_context(tile.TileContext(nc))
        pconst = ctx.enter_context(tc.tile_pool(name="const", bufs=1))
        poffs = ctx.enter_context(tc.tile_pool(name="offs", bufs=T))
        pq = ctx.enter_context(tc.tile_pool(name="q", bufs=T))
        pao = ctx.enter_context(tc.tile_pool(name="ao", bufs=T))

        ident = pconst.tile([128, 128], F32)
        make_identity(nc, ident[:])
        ones_row = pconst.tile([1, 128], F32)
        nc.vector.memset(ones_row[:], 1.0)
        bout_sb = pconst.tile([1, D], F32)
        nc.sync.dma_start(out=bout_sb[:], in_=bout[:])
        eps_sb = pconst.tile([128, 1], F32)
        nc.vector.memset(eps_sb[:], LN_EPS)

        offs_tiles = []
        for t in range(T):
            offs_t = poffs.tile([128, KN], I32)
            nc.sync.dma_start(out=offs_t[:], in_=idx[ts(t, 128), :])
            offs_tiles.append(offs_t)

        q_tiles = []
        ao_tiles = []

        # ---------------- Phase 1: projections + LN + KV shard ----------
        with (
            tc.tile_pool(name="xw", bufs=1) as pxw,
            tc.tile_pool(name="ps1", bufs=4, space="PSUM") as pps,
            tc.tile_pool(name="ln", bufs=4) as pln,
        ):
            xt_sb = []
            for dc in range(DCH):
                xt_c = pxw.tile([128, NSH], F32, tag=f"xt{dc}")
                nc.sync.dma_start(out=xt_c[:], in_=xT[ts(dc, 128), :])
                xt_sb.append(xt_c)
            w_sb = {}
            for wname, wdram in (("q", WqT), ("k", WkT), ("v", WvT)):
                w_sb[wname] = []
                for dc in range(DCH):
                    w_c = pxw.tile([128, D], F32, tag=f"w{wname}{dc}")
                    nc.sync.dma_start(out=w_c[:], in_=wdram[ts(dc, 128), :])
                    w_sb[wname].append(w_c)

            def layer_norm_from_psum(ps, out_bf):
                """Per-head LN of psum tile (128, D) -> bf16 SBUF tile."""
                ps_h = ps[:].rearrange("p (h d) -> p h d", h=H)
                sums = pln.tile([128, H], F32, tag="lnsum")
                nc.vector.tensor_reduce(
                    out=sums[:], in_=ps_h, axis=mybir.AxisListType.X,
                    op=mybir.AluOpType.add,
                )
                sq = pln.tile([128, D], F32, tag="lnsq")
                nc.scalar.square(out=sq[:], in_=ps[:])
                sqs = pln.tile([128, H], F32, tag="lnsqs")
                nc.vector.tensor_reduce(
                    out=sqs[:], in_=sq[:].rearrange("p (h d) -> p h d", h=H),
                    axis=mybir.AxisListType.X, op=mybir.AluOpType.add,
                )
                mu = pln.tile([128, H], F32, tag="lnmu")
                nc.vector.tensor_scalar_mul(mu[:], sums[:], 1.0 / DK)
                var = pln.tile([128, H], F32, tag="lnvar")
                # var = E[x^2] - mu^2   (E[x^2] = sqs/DK)
                nc.vector.tensor_scalar_mul(var[:], sqs[:], 1.0 / DK)
                musq = pln.tile([128, H], F32, tag="lnmusq")
                nc.vector.tensor_tensor(
                    out=musq[:], in0=mu[:], in1=mu[:], op=mybir.AluOpType.mult
                )
                nc.vector.tensor_tensor(
                    out=var[:], in0=var[:], in1=musq[:],
                    op=mybir.AluOpType.subtract,
                )
                std = pln.tile([128, H], F32, tag="lnstd")
                nc.scalar.activation(
                    out=std[:], in_=var[:],
                    func=mybir.ActivationFunctionType.Sqrt, bias=eps_sb[:],
                )
                rstd = pln.tile([128, H], F32, tag="lnrstd")
                nc.vector.reciprocal(rstd[:], std[:])
                cen = pln.tile([128, D], F32, tag="lncen")
                nc.vector.tensor_tensor(
                    out=cen[:].rearrange("p (h d) -> p h d", h=H),
                    in0=ps_h,
                    in1=mu[:].rearrange("p (h o) -> p h o", o=1)
                        .to_broadcast([128, H, DK]),
                    op=mybir.AluOpType.subtract,
                )
                nc.vector.tensor_tensor(
                    out=out_bf[:].rearrange("p (h d) -> p h d", h=H),
                    in0=cen[:].rearrange("p (h d) -> p h d", h=H),
                    in1=rstd[:].rearrange("p (h o) -> p h o", o=1)
                        .to_broadcast([128, H, DK]),
                    op=mybir.AluOpType.mult,
                )

            for t in range(T):
                for proj in ("q", "k", "v"):
                    ps = pps.tile([128, D], F32, tag="ps")
                    for dc in range(DCH):
                        nc.tensor.matmul(
                            out=ps[:],
                            lhsT=xt_sb[dc][:, ts(t, 128)],
                            rhs=w_sb[proj][dc][:],
                            start=(dc == 0),
                            stop=(dc == DCH - 1),
                        )
                    if proj == "q":
                        q_t = pq.tile([128, D], BF16)
                        layer_norm_from_psum(ps, q_t)
                        q_tiles.append(q_t)
                    elif proj == "k":
                        k_bf = pln.tile([128, D], BF16, tag="kbf")
                        layer_norm_from_psum(ps, k_bf)
                        nc.sync.dma_start(
                            out=kv_shard[ts(t, 128), 0:D], in_=k_bf[:]
                        )
                    else:
                        v_bf = pln.tile([128, D], BF16, tag="vbf")
                        nc.vector.tensor_copy(out=v_bf[:], in_=ps[:])
                        nc.sync.dma_start(
                            out=kv_shard[ts(t, 128), D:2 * D], in_=v_bf[:]
                        )

        # ---------------- AllGather K|V across the batch group ----------
        nc.gpsimd.collective_compute(
            "AllGather",
            mybir.AluOpType.bypass,
            replica_groups=groups,
            ins=[kv_shard[:]],
            outs=[kv_full[:]],
        )

        # ---------------- Phase 2: gather + scores + softmax + AV -------
        with (
            tc.tile_pool(name="kvg", bufs=2) as pkvg,
            tc.tile_pool(name="pbuf", bufs=3) as ppb,
            tc.tile_pool(name="sm", bufs=3) as psm,
        ):
            for t in range(T):
                offs_t = offs_tiles[t]
                kvg_g = []
                for g in range(G):
                    kvg = pkvg.tile([128, KG, 2 * D], BF16, tag="kvg")
                    for kk in range(KG):
                        nc.gpsimd.indirect_dma_start(
                            out=kvg[:, kk, :],
                            out_offset=None,
                            in_=kv_full[:],
                            in_offset=bass.IndirectOffsetOnAxis(
                                ap=offs_t[:, g * KG + kk: g * KG + kk + 1],
                                axis=0,
                            ),
                        )
                    kvg_g.append(kvg)

                sc = psm.tile([128, KN, H], F32, tag="sc")
                q_bc = (
                    q_tiles[t][:]
                    .rearrange("p (o h d) -> p o h d", o=1, h=H)
                    .to_broadcast([128, KG, H, DK])
                )
                for g in range(G):
                    pt = ppb.tile([128, KG, H, DK], BF16, tag="pbuf")
                    nc.vector.tensor_tensor(
                        out=pt[:],
                        in0=kvg_g[g][:, :, 0:D].rearrange(
                            "p k (h d) -> p k h d", h=H
                        ),
                        in1=q_bc,
                        op=mybir.AluOpType.mult,
                    )
                    nc.vector.tensor_reduce(
                        out=sc[:, g * KG:(g + 1) * KG, :],
                        in_=pt[:],
                        axis=mybir.AxisListType.X,
                        op=mybir.AluOpType.add,
                    )

                # softmax over k (scores bounded by ~8 after LN: skip max)
                es = psm.tile([128, KN, H], F32, tag="es")
                nc.scalar.activation(
                    out=es[:], in_=sc[:],
                    func=mybir.ActivationFunctionType.Exp,
                    scale=1.0 / float(np.sqrt(DK)),
                )
                ssum = psm.tile([128, H], F32, tag="ssum")
                nc.vector.tensor_reduce(
                    out=ssum[:], in_=es[:].rearrange("p k h -> p h k"),
                    axis=mybir.AxisListType.X, op=mybir.AluOpType.add,
                )
                rs = psm.tile([128, H], F32, tag="rs")
                nc.vector.reciprocal(rs[:], ssum[:])
                attn = psm.tile([128, KN, H], BF16, tag="attn")
                nc.vector.tensor_tensor(
                    out=attn[:],
                    in0=es[:],
                    in1=rs[:].rearrange("p (o h) -> p o h", o=1)
                        .to_broadcast([128, KN, H]),
                    op=mybir.AluOpType.mult,
                )

                ao_t = pao.tile([128, D], F32)
                ao_tiles.append(ao_t)
                for g in range(G):
                    p2 = ppb.tile([128, KG, H, DK], BF16, tag="pbuf")
                    nc.vector.tensor_tensor(
                        out=p2[:],
                        in0=kvg_g[g][:, :, D:2 * D].rearrange(
                            "p k (h d) -> p k h d", h=H
                        ),
                        in1=attn[:, g * KG:(g + 1) * KG, :]
                            .rearrange("p k (h o) -> p k h o", o=1)
                            .to_broadcast([128, KG, H, DK]),
                        op=mybir.AluOpType.mult,
                    )
                    m = KG // 2
                    while m > 1:
                        nc.vector.tensor_tensor(
                            out=p2[:, 0:m],
                            in0=p2[:, 0:m],
                            in1=p2[:, m:2 * m],
                            op=mybir.AluOpType.add,
                        )
                        m //= 2
                    av = psm.tile([128, H, DK], F32, tag="av")
                    nc.vector.tensor_tensor(
                        out=av[:].rearrange("p h d -> p (h d)")
                            .rearrange("p (o h d) -> p o h d", o=1, h=H),
                        in0=p2[:, 0:1],
                        in1=p2[:, 1:2],
                        op=mybir.AluOpType.add,
                    )
                    if g == 0:
                        nc.vector.tensor_copy(
                            out=ao_t[:], in_=av[:].rearrange("p h d -> p (h d)")
                        )
                    else:
                        nc.vector.tensor_tensor(
                            out=ao_t[:],
                            in0=ao_t[:],
                            in1=av[:].rearrange("p h d -> p (h d)"),
                            op=mybir.AluOpType.add,
                        )

        # ---------------- Phase 3: transpose + out-projection -----------
        with (
            tc.tile_pool(name="p3", bufs=1) as p3,
            tc.tile_pool(name="ps3", bufs=4, space="PSUM") as pps3,
            tc.tile_pool(name="pstr", bufs=4, space="PSUM") as pptr,
            tc.tile_pool(name="o3", bufs=3) as po3,
        ):
            wo_sb = []
            for dc in range(DCH):
                w_c = p3.tile([128, D], F32, tag=f"wo{dc}")
                nc.sync.dma_start(out=w_c[:], in_=WoT[ts(dc, 128), :])
                wo_sb.append(w_c)
            aot_sb = [
                p3.tile([128, NSH], F32, tag=f"aot{dc}", name=f"aot{dc}")
                for dc in range(DCH)
            ]
            for t in range(T):
                for dc in range(DCH):
                    tr_ps = pptr.tile([128, 128], F32, tag="tr")
                    nc.tensor.transpose(
                        out=tr_ps[:],
                        in_=ao_tiles[t][:, ts(dc, 128)],
                        identity=ident[:],
                    )
                    nc.vector.tensor_copy(
                        out=aot_sb[dc][:, ts(t, 128)], in_=tr_ps[:]
                    )
            for t in range(T):
                ps = pps3.tile([128, D], F32, tag="ps3")
                for dc in range(DCH):
                    nc.tensor.matmul(
                        out=ps[:],
                        lhsT=aot_sb[dc][:, ts(t, 128)],
                        rhs=wo_sb[dc][:],
                        start=(dc == 0),
                        stop=False,
                    )
                nc.tensor.matmul(
                    out=ps[:],
                    lhsT=ones_row[:],
                    rhs=bout_sb[:],
                    start=False,
                    stop=True,
                )
                o_sb = po3.tile([128, D], F32, tag="osb")
                nc.vector.tensor_copy(out=o_sb[:], in_=ps[:])
                nc.sync.dma_start(out=out[ts(t, 128), :], in_=o_sb[:])

    nc.finalize()
    return nc


_NC_CACHE = {}


def _get_nc(NB, NSH):
    key = (NB, NSH)
    if key not in _NC_CACHE:
        _NC_CACHE[key] = build_nc(NB, NSH)
    return _NC_CACHE[key]


def make_in_maps(x, idx, Wq, Wk, Wv, Wout, bout, NB, NSH):
    x = np.asarray(x, dtype=np.float32)
    idx = np.asarray(idx).astype(np.int32)
    WqT = np.ascontiguousarray(np.asarray(Wq, dtype=np.float32).T)
    WkT = np.ascontiguousarray(np.asarray(Wk, dtype=np.float32).T)
    WvT = np.ascontiguousarray(np.asarray(Wv, dtype=np.float32).T)
    WoT = np.ascontiguousarray(np.asarray(Wout, dtype=np.float32).T)
    bo = np.asarray(bout, dtype=np.float32).reshape(1, D)
    CPB = NCORES // B
    in_maps = []
    for c in range(NCORES):
        b, q = divmod(c, CPB)
        rows = slice(q * NSH, (q + 1) * NSH)
        in_maps.append({
            "xT_sh": np.ascontiguousarray(x[b, rows, :].T),
            "idx_sh": np.ascontiguousarray(idx[rows, :]),
            "WqT": WqT, "WkT": WkT, "WvT": WvT, "WoutT": WoT,
            "bout": bo,
        })
    return in_maps


def assemble(results, NB, NSH):
    CPB = NCORES // B
    out = np.empty((B, NB, D), dtype=np.float32)
    for c in range(NCORES):
        b, q = divmod(c, CPB)
        out[b, q * NSH:(q + 1) * NSH, :] = results[c]["out_sh"]
    return out


def kernel(x, idx, Wq, Wk, Wv, Wout, bout):
    from concourse.bass_utils import run_bass_kernel_spmd

    x = np.asarray(x)
    NB = x.shape[1]
    NSH = NB // (NCORES // B)
    nc = _get_nc(NB, NSH)
    in_maps = make_in_maps(x, idx, Wq, Wk, Wv, Wout, bout, NB, NSH)
    res = run_bass_kernel_spmd(nc, in_maps, list(range(NCORES)))
    return assemble(res.results, NB, NSH)
